# revision 22
# baseline (speedup 1.0000x reference)
"""Trainium2 Bass kernel for nn_A2CNetwork (GCN mean-pool + 2 MLP heads).

Self-contained: hardcodes shapes, shards batch over 8 NeuronCores
(2 graphs per core), builds one SPMD Bass/Tile graph, runs via
run_bass_kernel_spmd, reassembles full outputs.

Math (per graph), collapsing GCNConv + mean-pool algebraically:
  deg[i]  = 1 + #{e : dst_e == i}                (self-loops included)
  dis     = 1/sqrt(deg)
  t[j]    = sum_{e: src_e == j} dis[dst_e]
  u[j]    = dis[j] * (t[j] + dis[j]) / N
  v       = sum_j u[j] * gcn_x[j, :]             (weighted node sum)
  g       = v @ gcn_w + gcn_b
  h       = concat([g, x]);  two 3-layer MLP heads -> (a, c)

On device the histogram / gather / scatter run as one-hot matmuls over a
(hi=idx>>7, lo=idx&127) 80x128 decomposition: per 128-edge chunk, bf16
one-hot tiles are built with is_equal against iota tables, contracted on
the TensorEngine, and dis[dst_e] is extracted with a PE transpose + matmul
against dis2d^T, a masked multiply, and a batched reduce.
"""

import os
import sys

sys.path.insert(0, "/opt/trn_rl_repo")
os.environ.setdefault("MYCRO_LOCAL_CACHE", "1")

import numpy as np
import ml_dtypes

import concourse.bass as bass
import concourse.tile as tile
from concourse import mybir
from concourse.vector_clock import ScopedClock

# ----------------------------------------------------------------------------
# problem dims (hardcoded per spec)
B, N, E = 16, 10000, 160000
GCN_D, IN_DIM, OUT_DIM = 64, 128, 16
RAW_D = IN_DIM - GCN_D  # 64
H1, H2 = 512, 1024
N_CORES = 8
GPC = B // N_CORES  # graphs per core = 2

HI, LO = 80, 128  # node = hi*128 + lo;  hi in [0,79), lo in [0,128)
NPAD = HI * LO  # 10240
CHUNKS = E // 128  # 1250 edge chunks of 128 (edge k*? = strided layout)
EPP = E // 128  # elements per partition for edge arrays = 1250
BN = 16  # chunks per batched one-hot builder op

F32 = mybir.dt.float32
BF16 = mybir.dt.bfloat16
I32 = mybir.dt.int32
AF = mybir.ActivationFunctionType
OP = mybir.AluOpType

DEBUG = False




class SplitDrainTileContext(tile.TileContext):
    """Walrus in this image rejects >1 sync-wait on one Drain; split them."""

    MAX_WAITS = 1

    def _drain_and_barrier(self, tick_clock, wait_clock):
        import bass_rust

        drain_inst = self.nc.sync.drain()
        wait_clock.add_sem_waits(
            drain_inst.ins, ScopedClock({None: tick_clock.global_clock})
        )
        si = drain_inst.ins.sync_info
        mw = self.MAX_WAITS
        if si is not None and si.on_wait is not None and len(si.on_wait) > mw:
            waits = list(si.on_wait)
            si.on_wait = waits[:mw]
            rest = waits[mw:]
            while rest:
                d2 = self.nc.sync.drain()
                if d2.ins.sync_info is None:
                    d2.ins.sync_info = bass_rust.SyncInfo(
                        on_wait=rest[:mw], on_update=[]
                    )
                else:
                    d2.ins.sync_info.on_wait = rest[:mw]
                rest = rest[mw:]
        self.nc.all_engine_barrier()
        assert self.sems is not None
        popped = self.nc._tile_sem_poison_stack.pop()
        assert popped is self._sem_poison
        self.nc.clear_and_free_semaphores(list(self.sems.allocated().values()))
        self.nc.all_engine_barrier()


def _ap3(ap, steps):
    """Manual 3D AP on top of a 2D slice: steps = [[s1,c1],[s2,c2]] free dims."""
    return bass.AP(ap.tensor, ap.offset, [list(ap.ap[0])] + [list(s) for s in steps])


def split_waits(nc, max_waits=1):
    """This walrus build encodes at most one sync-wait per instruction.

    For any instruction carrying more, move the extras onto same-engine NOPs
    inserted immediately before it (same sequencer => still ordered).
    """
    import bass_rust

    for bb in nc.main_func.blocks:
        il = bb.instructions
        extra_total = 0
        newl = []
        for ins in il:
            si = ins.sync_info
            if si is not None and si.on_wait is not None and len(si.on_wait) > max_waits:
                waits = list(si.on_wait)
                si.on_wait = waits[:max_waits]
                rest = waits[max_waits:]
                eng = nc.engines[ins.engine]
                while rest:
                    nop = eng.nop()
                    # relocate: eng.nop() appended to the current bb tail
                    src_list = nc.cur_bb.bb.instructions
                    assert src_list[-1] is nop.ins
                    src_list.pop()
                    nop.ins.sync_info = bass_rust.SyncInfo(
                        on_wait=rest[:max_waits], on_update=[]
                    )
                    newl.append(nop.ins)
                    rest = rest[max_waits:]
                    extra_total += 1
            newl.append(ins)
        il[:] = newl


def build_nc(chunks=CHUNKS, debug=DEBUG):
    nc = bass.Bass()

    # ---- dram parameters (per-core shards / replicated weights / constants)
    edges = nc.declare_dram_parameter("edges", [GPC, 2, E], I32, isOutput=False)
    gx = nc.declare_dram_parameter("gx", [GPC, HI, 128, GCN_D], F32, isOutput=False)
    xT = nc.declare_dram_parameter("xT", [RAW_D, GPC], F32, isOutput=False)
    gcn_w = nc.declare_dram_parameter("gcn_w", [GCN_D, GCN_D], F32, isOutput=False)
    gcn_b = nc.declare_dram_parameter("gcn_b", [GCN_D, 1], F32, isOutput=False)
    aw1 = nc.declare_dram_parameter("aw1", [IN_DIM, H1], F32, isOutput=False)
    aw2 = nc.declare_dram_parameter("aw2", [4, 128, H2], F32, isOutput=False)
    aw3 = nc.declare_dram_parameter("aw3", [8, 128, OUT_DIM], F32, isOutput=False)
    cw1 = nc.declare_dram_parameter("cw1", [IN_DIM, H1], F32, isOutput=False)
    cw2 = nc.declare_dram_parameter("cw2", [4, 128, H2], F32, isOutput=False)
    cw3 = nc.declare_dram_parameter("cw3", [8, 128, 1], F32, isOutput=False)
    ab1 = nc.declare_dram_parameter("ab1", [128, 4], F32, isOutput=False)
    ab2 = nc.declare_dram_parameter("ab2", [128, 8], F32, isOutput=False)
    ab3 = nc.declare_dram_parameter("ab3", [OUT_DIM, 1], F32, isOutput=False)
    cb1 = nc.declare_dram_parameter("cb1", [128, 4], F32, isOutput=False)
    cb2 = nc.declare_dram_parameter("cb2", [128, 8], F32, isOutput=False)
    cb3 = nc.declare_dram_parameter("cb3", [1, 1], F32, isOutput=False)
    iota128 = nc.declare_dram_parameter("iota128", [128, 128], BF16, isOutput=False)
    iota80 = nc.declare_dram_parameter("iota80", [128, HI], BF16, isOutput=False)
    ident_bf = nc.declare_dram_parameter("ident_bf", [128, 128], BF16, isOutput=False)
    out = nc.declare_dram_parameter("out", [OUT_DIM + 1, GPC], F32, isOutput=True)
    if debug:
        dbg = nc.declare_dram_parameter("dbg", [128, 256], F32, isOutput=True)

    with SplitDrainTileContext(nc) as tc:
        with (
            tc.tile_pool(name="const", bufs=1) as constp,
            tc.tile_pool(name="edge", bufs=2) as edgep,
            tc.tile_pool(name="oneh", bufs=2) as onehp,
            tc.tile_pool(name="gxp", bufs=1) as gxp,
            tc.tile_pool(name="small", bufs=2) as smallp,
            tc.tile_pool(name="mlp", bufs=1) as mlpp,
            tc.tile_pool(name="acc", bufs=1, space="PSUM") as accp,
            tc.tile_pool(name="pch", bufs=2, space="PSUM") as pchp,
            tc.tile_pool(name="pmlp", bufs=2, space="PSUM") as pmlpp,
            tc.tile_pool(name="dram", bufs=2, space="DRAM") as dramp,
        ):
            # ---- constants + weights to SBUF
            iota128_sb = constp.tile([128, 128], BF16, tag="iota128")
            nc.sync.dma_start(iota128_sb[:], iota128[:])
            iota80_sb = constp.tile([128, HI], BF16, tag="iota80")
            nc.sync.dma_start(iota80_sb[:], iota80[:])
            ident_sb = constp.tile([128, 128], BF16, tag="ident")
            nc.sync.dma_start(ident_sb[:], ident_bf[:])

            gcnw_sb = constp.tile([GCN_D, GCN_D], F32, tag="gcnw")
            nc.sync.dma_start(gcnw_sb[:], gcn_w[:])
            gcnb_sb = constp.tile([GCN_D, 1], F32, tag="gcnb")
            nc.sync.dma_start(gcnb_sb[:], gcn_b[:])

            w1_sb = {}
            w2_sb = {}
            w3_sb = {}
            b1_sb = {}
            b2_sb = {}
            b3_sb = {}
            for nm, (W1, W2, W3, B1, B2, B3, od) in {
                "a": (aw1, aw2, aw3, ab1, ab2, ab3, OUT_DIM),
                "c": (cw1, cw2, cw3, cb1, cb2, cb3, 1),
            }.items():
                w1_sb[nm] = constp.tile([128, H1], F32, name=f"w1_sb_{nm}", tag=f"{nm}w1")
                nc.sync.dma_start(w1_sb[nm][:], W1[:])
                w2_sb[nm] = constp.tile([128, 4 * H2], F32, name=f"w2_sb_{nm}", tag=f"{nm}w2")
                nc.sync.dma_start(w2_sb[nm][:].rearrange("p (a m) -> p a m", a=4), W2[:].rearrange("a p m -> p a m"))
                w3_sb[nm] = constp.tile([128, 8 * od], F32, name=f"w3_sb_{nm}", tag=f"{nm}w3")
                nc.sync.dma_start(w3_sb[nm][:].rearrange("p (a m) -> p a m", a=8), W3[:].rearrange("a p m -> p a m"))
                b1_sb[nm] = constp.tile([128, 4], F32, name=f"b1_sb_{nm}", tag=f"{nm}b1")
                nc.sync.dma_start(b1_sb[nm][:], B1[:])
                b2_sb[nm] = constp.tile([128, 8], F32, name=f"b2_sb_{nm}", tag=f"{nm}b2")
                nc.sync.dma_start(b2_sb[nm][:], B2[:])
                b3_sb[nm] = constp.tile([od, 1], F32, name=f"b3_sb_{nm}", tag=f"{nm}b3")
                nc.sync.dma_start(b3_sb[nm][:], B3[:])

            hT_sb = mlpp.tile([128, GPC], F32, tag="hT")
            nc.sync.dma_start(hT_sb[GCN_D:128, :], xT[:])

            if debug:
                dbg_sb = mlpp.tile([128, 256], F32, tag="dbg")

            def onehot(out_t, bn, vals2d, iota_sb, width, eng=None):
                """out_t[:, :bn*width] = onehot(vals2d[:, :bn]) against iota row."""
                vals_ap = _ap3(vals2d, [[1, bn], [0, width]])
                iota_ap = _ap3(iota_sb[:, 0:width], [[0, bn], [1, width]])
                out_ap = _ap3(
                    out_t[:, 0 : bn * width], [[width, bn], [1, width]]
                )
                (eng or nc.vector).tensor_tensor(
                    out=out_ap, in0=vals_ap, in1=iota_ap, op=OP.is_equal
                )

            # ================= per-graph pipeline =================
            for g in range(GPC):
                # ---- load edges, split hi/lo, cast to bf16
                dst_i = edgep.tile([128, EPP], I32, tag="dst_i")
                nc.sync.dma_start(
                    dst_i[:], edges[g, 1].rearrange("(p c) -> p c", p=128)
                )
                src_i = edgep.tile([128, EPP], I32, tag="src_i")
                nc.sync.dma_start(
                    src_i[:], edges[g, 0].rearrange("(p c) -> p c", p=128)
                )

                tmp_i = edgep.tile([128, EPP], I32, tag="tmp_i")
                dhi = edgep.tile([128, EPP], BF16, tag="dhi")
                dlo = edgep.tile([128, EPP], BF16, tag="dlo")
                shi = edgep.tile([128, EPP], BF16, tag="shi")
                slo = edgep.tile([128, EPP], BF16, tag="slo")
                for (idx_t, hi_t, lo_t) in ((dst_i, dhi, dlo), (src_i, shi, slo)):
                    nc.vector.tensor_scalar(
                        out=tmp_i[:], in0=idx_t[:], scalar1=7, scalar2=None,
                        op0=OP.arith_shift_right,
                    )
                    nc.vector.tensor_copy(out=hi_t[:], in_=tmp_i[:])
                    nc.vector.tensor_scalar(
                        out=tmp_i[:], in0=idx_t[:], scalar1=127, scalar2=None,
                        op0=OP.bitwise_and,
                    )
                    nc.vector.tensor_copy(out=lo_t[:], in_=tmp_i[:])

                # ---- load this graph's node features (overlaps edge phase)
                gx_sb = gxp.tile([128, HI * GCN_D], F32, tag="gx")
                nc.sync.dma_start(gx_sb[:].rearrange("p (c d) -> p c d", c=HI), gx[g].rearrange("c p d -> p c d"))

                # ---- phase B: degree histogram  deg2dT[lo, hi]
                bt_dram = dramp.tile([chunks, 128, 128], BF16, tag="btd", name="bt_dram")
                ad_dram = dramp.tile([chunks, 128, HI], BF16, tag="add", name="ad_dram")
                degT_ps = accp.tile([128, HI], F32, tag="degT", space="PSUM")
                k = 0
                while k < chunks:
                    bn = min(BN, chunks - k)
                    Bd = onehp.tile([128, BN * 128], BF16, tag="Bd")
                    Ad = onehp.tile([128, BN * HI], BF16, tag="Ad")
                    onehot(Bd, bn, dlo[:, k : k + bn], iota128_sb, 128)
                    onehot(Ad, bn, dhi[:, k : k + bn], iota80_sb, HI)
                    for j in range(bn):
                        nc.tensor.matmul(
                            out=degT_ps[:],
                            lhsT=Bd[:, j * 128 : (j + 1) * 128],
                            rhs=Ad[:, j * HI : (j + 1) * HI],
                            start=(k + j == 0),
                            stop=(k + j == chunks - 1),
                            skip_group_check=True,
                        )
                    # transpose Bd now and spill BT + Ad to DRAM scratch so
                    # phase D streams them back instead of rebuilding (DVE is
                    # the bottleneck; DMA engines are idle)
                    for j0 in range(0, bn, 4):
                        b4 = min(4, bn - j0)
                        BT_ps = pchp.tile([128, 4 * 128], BF16, tag="BT", space="PSUM")
                        BT_sb = onehp.tile([128, 4 * 128], BF16, tag="BTsb")
                        for q in range(b4):
                            j = j0 + q
                            nc.tensor.transpose(
                                out=BT_ps[:, q * 128 : (q + 1) * 128],
                                in_=Bd[:, j * 128 : (j + 1) * 128],
                                identity=ident_sb[:],
                            )
                        nc.scalar.activation(
                            out=BT_sb[:, 0 : b4 * 128], in_=BT_ps[:, 0 : b4 * 128],
                            func=AF.Identity,
                        )
                        nc.sync.dma_start(
                            bt_dram[k + j0 : k + j0 + b4].rearrange("q p l -> p q l"),
                            BT_sb[:, 0 : b4 * 128].rearrange("p (q l) -> p q l", q=b4),
                        )
                    nc.sync.dma_start(
                        ad_dram[k : k + bn].rearrange("q p h -> p q h"),
                        Ad[:, 0 : bn * HI].rearrange("p (q h) -> p q h", q=bn),
                    )
                    k += bn

                # ---- phase C: dis = 1/sqrt(deg+1); keep f32 + bf16 copies
                deg1_sb = smallp.tile([128, HI], F32, tag="deg1")
                nc.scalar.activation(
                    out=deg1_sb[:], in_=degT_ps[:], func=AF.Identity, bias=1.0
                )
                rec_sb = smallp.tile([128, HI], F32, tag="rec")
                nc.vector.reciprocal(out=rec_sb[:], in_=deg1_sb[:])
                disT_sb = smallp.tile([128, HI], F32, tag="disT")
                nc.scalar.activation(out=disT_sb[:], in_=rec_sb[:], func=AF.Sqrt)
                disT_bf = smallp.tile([128, HI], BF16, tag="disTbf")
                nc.vector.tensor_copy(out=disT_bf[:], in_=disT_sb[:])

                # ---- phase D: dis_dst gather + t scatter  t2dT[lo, hi]
                tT_ps = accp.tile([128, HI], F32, tag="tT", space="PSUM")
                dd_f = edgep.tile([128, EPP], F32, tag="dd_f")
                k = 0
                while k < chunks:
                    bn = min(BN, chunks - k)
                    Bs = onehp.tile([128, BN * 128], BF16, tag="Bs")
                    WA = onehp.tile([128, BN * HI], BF16, tag="WA")
                    scr = onehp.tile([128, BN * HI], BF16, tag="scr")
                    G_bt = onehp.tile([128, BN * HI], BF16, tag="Gbt")
                    onehot(Bs, bn, slo[:, k : k + bn], iota128_sb, 128)
                    # stream BT back from scratch, run G matmuls 4 chunks at a time
                    for j0 in range(0, bn, 4):
                        b4 = min(4, bn - j0)
                        G_ps = pchp.tile([128, 4 * HI], F32, tag="G", space="PSUM")
                        BT_sb = onehp.tile([128, 4 * 128], BF16, tag="BTsb")
                        nc.sync.dma_start(
                            BT_sb[:, 0 : b4 * 128].rearrange("p (q l) -> p q l", q=b4),
                            bt_dram[k + j0 : k + j0 + b4].rearrange("q p l -> p q l"),
                        )
                        for q in range(b4):
                            nc.tensor.matmul(
                                out=G_ps[:, q * HI : (q + 1) * HI],
                                lhsT=BT_sb[:, q * 128 : (q + 1) * 128],
                                rhs=disT_bf[:],
                                skip_group_check=True,
                            )
                        nc.scalar.activation(
                            out=G_bt[:, j0 * HI : (j0 + b4) * HI],
                            in_=G_ps[:, 0 : b4 * HI],
                            func=AF.Identity,
                        )
                    Ad = onehp.tile([128, BN * HI], BF16, tag="Ad")
                    As = onehp.tile([128, BN * HI], BF16, tag="As")
                    nc.sync.dma_start(
                        Ad[:, 0 : bn * HI].rearrange("p (q h) -> p q h", q=bn),
                        ad_dram[k : k + bn].rearrange("q p h -> p q h"),
                    )
                    onehot(As, bn, shi[:, k : k + bn], iota80_sb, HI)
                    nc.vector.tensor_tensor(
                        out=scr[:, 0 : bn * HI], in0=Ad[:, 0 : bn * HI],
                        in1=G_bt[:, 0 : bn * HI], op=OP.mult,
                    )
                    nc.vector.tensor_reduce(
                        out=dd_f[:, k : k + bn],
                        in_=_ap3(scr[:, 0 : bn * HI], [[HI, bn], [1, HI]]),
                        axis=mybir.AxisListType.X,
                        op=OP.add,
                    )
                    nc.vector.tensor_tensor(
                        out=_ap3(WA[:, 0 : bn * HI], [[HI, bn], [1, HI]]),
                        in0=_ap3(As[:, 0 : bn * HI], [[HI, bn], [1, HI]]),
                        in1=_ap3(dd_f[:, k : k + bn], [[1, bn], [0, HI]]),
                        op=OP.mult,
                    )
                    for j in range(bn):
                        nc.tensor.matmul(
                            out=tT_ps[:],
                            lhsT=Bs[:, j * 128 : (j + 1) * 128],
                            rhs=WA[:, j * HI : (j + 1) * HI],
                            start=(k + j == 0),
                            stop=(k + j == chunks - 1),
                            skip_group_check=True,
                        )
                    k += bn

                # ---- phase E: u, weighted node-sum v, g
                uT_sb = smallp.tile([128, HI], F32, tag="uT")
                nc.vector.tensor_tensor(
                    out=uT_sb[:], in0=tT_ps[:], in1=disT_sb[:], op=OP.add
                )
                nc.vector.tensor_tensor(
                    out=uT_sb[:], in0=uT_sb[:], in1=disT_sb[:], op=OP.mult
                )
                nc.vector.tensor_scalar(
                    out=uT_sb[:], in0=uT_sb[:], scalar1=1.0 / N, scalar2=None,
                    op0=OP.mult,
                )

                vT_ps = pmlpp.tile([GCN_D, 1], F32, tag="mp", space="PSUM")
                for h in range(HI):
                    nc.tensor.matmul(
                        out=vT_ps[:],
                        lhsT=gx_sb[:, h * GCN_D : (h + 1) * GCN_D],
                        rhs=uT_sb[:, h : h + 1],
                        start=(h == 0),
                        stop=(h == HI - 1),
                        skip_group_check=True,
                    )
                vT_sb = smallp.tile([GCN_D, 1], F32, tag="vT_sb")
                nc.scalar.activation(out=vT_sb[:], in_=vT_ps[:], func=AF.Identity)
                gT_ps = pmlpp.tile([GCN_D, 1], F32, tag="mp", space="PSUM")
                nc.tensor.matmul(
                    out=gT_ps[:], lhsT=gcnw_sb[:], rhs=vT_sb[:],
                    skip_group_check=True,
                )
                nc.scalar.activation(
                    out=hT_sb[0:GCN_D, g : g + 1], in_=gT_ps[:],
                    func=AF.Identity, bias=gcnb_sb[:, 0:1],
                )

                if debug:
                    if g == 0:
                        nc.scalar.activation(
                            out=dbg_sb[:, 0:HI], in_=deg1_sb[:], func=AF.Identity
                        )
                        nc.scalar.activation(
                            out=dbg_sb[:, 80:160], in_=uT_sb[:], func=AF.Identity
                        )
                        nc.scalar.activation(
                            out=dbg_sb[:, 160:240], in_=disT_sb[:], func=AF.Identity
                        )
                        nc.scalar.activation(
                            out=dbg_sb[:, 240:241], in_=dd_f[:, 0:1], func=AF.Identity
                        )

            # ================= MLP heads (both graphs batched) =================
            def lrelu(out_sb, in_ps, bias_ap):
                nc.scalar.activation(
                    out=out_sb, in_=in_ps, func=AF.Identity, bias=bias_ap
                )
                tmp = mlpp.tile([128, GPC], F32, tag="lrtmp")
                nc.vector.tensor_scalar(
                    out=tmp[: out_sb.shape[0], :], in0=out_sb, scalar1=0.01,
                    scalar2=None, op0=OP.mult,
                )
                nc.vector.tensor_tensor(
                    out=out_sb, in0=out_sb, in1=tmp[: out_sb.shape[0], :], op=OP.max
                )

            outT_parts = {}
            for nm, od in (("a", OUT_DIM), ("c", 1)):
                a1_ps = pmlpp.tile([128, 4 * GPC], F32, tag="mp", space="PSUM")
                for m in range(4):
                    nc.tensor.matmul(
                        out=a1_ps[:, m * GPC : (m + 1) * GPC],
                        lhsT=w1_sb[nm][:, m * 128 : (m + 1) * 128],
                        rhs=hT_sb[:],
                        skip_group_check=True,
                    )
                a1_sb = mlpp.tile([128, 4 * GPC], F32, tag=f"{nm}1sb")
                for m in range(4):
                    lrelu(
                        a1_sb[:, m * GPC : (m + 1) * GPC],
                        a1_ps[:, m * GPC : (m + 1) * GPC],
                        b1_sb[nm][:, m : m + 1],
                    )
                a2_ps = pmlpp.tile([128, 8 * GPC], F32, tag="mp", space="PSUM")
                for m in range(8):
                    for kk in range(4):
                        nc.tensor.matmul(
                            out=a2_ps[:, m * GPC : (m + 1) * GPC],
                            lhsT=w2_sb[nm][:, kk * H2 + m * 128 : kk * H2 + (m + 1) * 128],
                            rhs=a1_sb[:, kk * GPC : (kk + 1) * GPC],
                            start=(kk == 0),
                            stop=(kk == 3),
                            skip_group_check=True,
                        )
                a2_sb = mlpp.tile([128, 8 * GPC], F32, tag=f"{nm}2sb")
                for m in range(8):
                    lrelu(
                        a2_sb[:, m * GPC : (m + 1) * GPC],
                        a2_ps[:, m * GPC : (m + 1) * GPC],
                        b2_sb[nm][:, m : m + 1],
                    )
                a3_ps = pmlpp.tile([od, GPC], F32, tag="mp", space="PSUM")
                for kk in range(8):
                    nc.tensor.matmul(
                        out=a3_ps[:],
                        lhsT=w3_sb[nm][:, kk * od : (kk + 1) * od],
                        rhs=a2_sb[:, kk * GPC : (kk + 1) * GPC],
                        start=(kk == 0),
                        stop=(kk == 7),
                        skip_group_check=True,
                    )
                a3_sb = mlpp.tile([od, GPC], F32, tag=f"{nm}3sb")
                nc.scalar.activation(
                    out=a3_sb[:], in_=a3_ps[:], func=AF.Identity,
                    bias=b3_sb[nm][:, 0:1],
                )
                outT_parts[nm] = a3_sb

            nc.sync.dma_start(out[0:OUT_DIM, :], outT_parts["a"][:])
            nc.sync.dma_start(out[OUT_DIM : OUT_DIM + 1, :], outT_parts["c"][:])
            if debug:
                nc.sync.dma_start(dbg[:], dbg_sb[:])

    split_waits(nc)
    return nc


_NC_CACHE = {}


def _get_nc(chunks=CHUNKS, debug=DEBUG):
    key = (chunks, debug)
    if key not in _NC_CACHE:
        _NC_CACHE[key] = build_nc(chunks, debug)
    return _NC_CACHE[key]


def make_in_maps(x, gcn_x, gcn_edge_index, gcn_w, gcn_b,
                 aw1, ab1, aw2, ab2, aw3, ab3,
                 cw1, cb1, cw2, cb2, cw3, cb3):
    f32 = np.float32
    bf = ml_dtypes.bfloat16
    gxp = np.zeros((B, NPAD, GCN_D), f32)
    gxp[:, :N, :] = np.asarray(gcn_x, f32)
    gxp = gxp.reshape(B, HI, 128, GCN_D)

    shared = {
        "gcn_w": np.ascontiguousarray(np.asarray(gcn_w, f32)),
        "gcn_b": np.asarray(gcn_b, f32).reshape(GCN_D, 1),
        "aw1": np.ascontiguousarray(np.asarray(aw1, f32)),
        "aw2": np.ascontiguousarray(np.asarray(aw2, f32).reshape(4, 128, H2)),
        "aw3": np.ascontiguousarray(np.asarray(aw3, f32).reshape(8, 128, OUT_DIM)),
        "cw1": np.ascontiguousarray(np.asarray(cw1, f32)),
        "cw2": np.ascontiguousarray(np.asarray(cw2, f32).reshape(4, 128, H2)),
        "cw3": np.ascontiguousarray(np.asarray(cw3, f32).reshape(8, 128, 1)),
        "ab1": np.ascontiguousarray(np.asarray(ab1, f32).reshape(4, 128).T),
        "ab2": np.ascontiguousarray(np.asarray(ab2, f32).reshape(8, 128).T),
        "ab3": np.asarray(ab3, f32).reshape(OUT_DIM, 1),
        "cb1": np.ascontiguousarray(np.asarray(cb1, f32).reshape(4, 128).T),
        "cb2": np.ascontiguousarray(np.asarray(cb2, f32).reshape(8, 128).T),
        "cb3": np.asarray(cb3, f32).reshape(1, 1),
        "iota128": np.ascontiguousarray(
            np.broadcast_to(np.arange(128, dtype=bf), (128, 128))
        ),
        "iota80": np.ascontiguousarray(
            np.broadcast_to(np.arange(HI, dtype=bf), (128, HI))
        ),
        "ident_bf": np.eye(128, dtype=bf),
    }
    in_maps = []
    for c in range(N_CORES):
        sl = slice(c * GPC, (c + 1) * GPC)
        m = dict(shared)
        m["edges"] = np.ascontiguousarray(np.asarray(gcn_edge_index[sl], np.int32))
        m["gx"] = np.ascontiguousarray(gxp[sl])
        m["xT"] = np.ascontiguousarray(np.asarray(x[sl], f32).T)
        in_maps.append(m)
    return in_maps


def run(inputs, trace=False, chunks=CHUNKS, debug=DEBUG):
    from concourse.bass_utils import run_bass_kernel_spmd

    nc = _get_nc(chunks, debug)
    in_maps = make_in_maps(**inputs)
    res = run_bass_kernel_spmd(
        nc, in_maps, core_ids=list(range(N_CORES)), trace=trace
    )
    a = np.zeros((B, OUT_DIM), np.float32)
    cc = np.zeros((B, 1), np.float32)
    for i in range(N_CORES):
        o = res.results[i]["out"]  # [17, GPC]
        a[i * GPC : (i + 1) * GPC] = o[:OUT_DIM].T
        cc[i * GPC : (i + 1) * GPC] = o[OUT_DIM:].T
    return (a, cc), res


def kernel(**inputs):
    (a, cc), _ = run(inputs, trace=False)
    return (a, cc)


# revision 23
# speedup vs baseline: 1.1098x; 1.1098x over previous
"""Trainium2 Bass kernel for nn_A2CNetwork (GCN mean-pool + 2 MLP heads).

Self-contained: hardcodes shapes, shards batch over 8 NeuronCores
(2 graphs per core), builds one SPMD Bass/Tile graph, runs via
run_bass_kernel_spmd, reassembles full outputs.

Math (per graph), collapsing GCNConv + mean-pool algebraically:
  deg[i]  = 1 + #{e : dst_e == i}                (self-loops included)
  dis     = 1/sqrt(deg)
  t[j]    = sum_{e: src_e == j} dis[dst_e]
  u[j]    = dis[j] * (t[j] + dis[j]) / N
  v       = sum_j u[j] * gcn_x[j, :]             (weighted node sum)
  g       = v @ gcn_w + gcn_b
  h       = concat([g, x]);  two 3-layer MLP heads -> (a, c)

On device the histogram / gather / scatter run as one-hot matmuls over a
(hi=idx>>7, lo=idx&127) 80x128 decomposition: per 128-edge chunk, bf16
one-hot tiles are built with is_equal against iota tables, contracted on
the TensorEngine, and dis[dst_e] is extracted with a PE transpose + matmul
against dis2d^T, a masked multiply, and a batched reduce.
"""

import os
import sys

sys.path.insert(0, "/opt/trn_rl_repo")
os.environ.setdefault("MYCRO_LOCAL_CACHE", "1")

import numpy as np
import ml_dtypes

import concourse.bass as bass
import concourse.tile as tile
from concourse import mybir
from concourse.vector_clock import ScopedClock

# ----------------------------------------------------------------------------
# problem dims (hardcoded per spec)
B, N, E = 16, 10000, 160000
GCN_D, IN_DIM, OUT_DIM = 64, 128, 16
RAW_D = IN_DIM - GCN_D  # 64
H1, H2 = 512, 1024
N_CORES = 8
GPC = B // N_CORES  # graphs per core = 2

HI, LO = 80, 128  # node = hi*128 + lo;  hi in [0,79), lo in [0,128)
NPAD = HI * LO  # 10240
CHUNKS = E // 128  # 1250 edge chunks of 128 (edge k*? = strided layout)
EPP = E // 128  # elements per partition for edge arrays = 1250
BN = 16  # chunks per batched one-hot builder op

F32 = mybir.dt.float32
BF16 = mybir.dt.bfloat16
I32 = mybir.dt.int32
AF = mybir.ActivationFunctionType
OP = mybir.AluOpType

DEBUG = False




class SplitDrainTileContext(tile.TileContext):
    """Walrus in this image rejects >1 sync-wait on one Drain; split them."""

    MAX_WAITS = 1

    def _drain_and_barrier(self, tick_clock, wait_clock):
        import bass_rust

        drain_inst = self.nc.sync.drain()
        wait_clock.add_sem_waits(
            drain_inst.ins, ScopedClock({None: tick_clock.global_clock})
        )
        si = drain_inst.ins.sync_info
        mw = self.MAX_WAITS
        if si is not None and si.on_wait is not None and len(si.on_wait) > mw:
            waits = list(si.on_wait)
            si.on_wait = waits[:mw]
            rest = waits[mw:]
            while rest:
                d2 = self.nc.sync.drain()
                if d2.ins.sync_info is None:
                    d2.ins.sync_info = bass_rust.SyncInfo(
                        on_wait=rest[:mw], on_update=[]
                    )
                else:
                    d2.ins.sync_info.on_wait = rest[:mw]
                rest = rest[mw:]
        self.nc.all_engine_barrier()
        assert self.sems is not None
        popped = self.nc._tile_sem_poison_stack.pop()
        assert popped is self._sem_poison
        self.nc.clear_and_free_semaphores(list(self.sems.allocated().values()))
        self.nc.all_engine_barrier()


def _ap3(ap, steps):
    """Manual 3D AP on top of a 2D slice: steps = [[s1,c1],[s2,c2]] free dims."""
    return bass.AP(ap.tensor, ap.offset, [list(ap.ap[0])] + [list(s) for s in steps])


def split_waits(nc, max_waits=1):
    """This walrus build encodes at most one sync-wait per instruction.

    For any instruction carrying more, move the extras onto same-engine NOPs
    inserted immediately before it (same sequencer => still ordered).
    """
    import bass_rust

    for bb in nc.main_func.blocks:
        il = bb.instructions
        extra_total = 0
        newl = []
        for ins in il:
            si = ins.sync_info
            if si is not None and si.on_wait is not None and len(si.on_wait) > max_waits:
                waits = list(si.on_wait)
                si.on_wait = waits[:max_waits]
                rest = waits[max_waits:]
                eng = nc.engines[ins.engine]
                while rest:
                    nop = eng.nop()
                    # relocate: eng.nop() appended to the current bb tail
                    src_list = nc.cur_bb.bb.instructions
                    assert src_list[-1] is nop.ins
                    src_list.pop()
                    nop.ins.sync_info = bass_rust.SyncInfo(
                        on_wait=rest[:max_waits], on_update=[]
                    )
                    newl.append(nop.ins)
                    rest = rest[max_waits:]
                    extra_total += 1
            newl.append(ins)
        il[:] = newl


def build_nc(chunks=CHUNKS, debug=DEBUG):
    nc = bass.Bass()

    # ---- dram parameters (per-core shards / replicated weights / constants)
    edges = nc.declare_dram_parameter("edges", [GPC, 2, E], I32, isOutput=False)
    gx = nc.declare_dram_parameter("gx", [GPC, HI, 128, GCN_D], F32, isOutput=False)
    xT = nc.declare_dram_parameter("xT", [RAW_D, GPC], F32, isOutput=False)
    gcn_w = nc.declare_dram_parameter("gcn_w", [GCN_D, GCN_D], F32, isOutput=False)
    gcn_b = nc.declare_dram_parameter("gcn_b", [GCN_D, 1], F32, isOutput=False)
    aw1 = nc.declare_dram_parameter("aw1", [IN_DIM, H1], F32, isOutput=False)
    aw2 = nc.declare_dram_parameter("aw2", [4, 128, H2], F32, isOutput=False)
    aw3 = nc.declare_dram_parameter("aw3", [8, 128, OUT_DIM], F32, isOutput=False)
    cw1 = nc.declare_dram_parameter("cw1", [IN_DIM, H1], F32, isOutput=False)
    cw2 = nc.declare_dram_parameter("cw2", [4, 128, H2], F32, isOutput=False)
    cw3 = nc.declare_dram_parameter("cw3", [8, 128, 1], F32, isOutput=False)
    ab1 = nc.declare_dram_parameter("ab1", [128, 4], F32, isOutput=False)
    ab2 = nc.declare_dram_parameter("ab2", [128, 8], F32, isOutput=False)
    ab3 = nc.declare_dram_parameter("ab3", [OUT_DIM, 1], F32, isOutput=False)
    cb1 = nc.declare_dram_parameter("cb1", [128, 4], F32, isOutput=False)
    cb2 = nc.declare_dram_parameter("cb2", [128, 8], F32, isOutput=False)
    cb3 = nc.declare_dram_parameter("cb3", [1, 1], F32, isOutput=False)
    iota128 = nc.declare_dram_parameter("iota128", [128, 128], BF16, isOutput=False)
    iota80 = nc.declare_dram_parameter("iota80", [128, HI], BF16, isOutput=False)
    ident_bf = nc.declare_dram_parameter("ident_bf", [128, 128], BF16, isOutput=False)
    out = nc.declare_dram_parameter("out", [OUT_DIM + 1, GPC], F32, isOutput=True)
    if debug:
        dbg = nc.declare_dram_parameter("dbg", [128, 256], F32, isOutput=True)

    with SplitDrainTileContext(nc) as tc:
        with (
            tc.tile_pool(name="const", bufs=1) as constp,
            tc.tile_pool(name="edge", bufs=2) as edgep,
            tc.tile_pool(name="oneh", bufs=2) as onehp,
            tc.tile_pool(name="gxp", bufs=1) as gxp,
            tc.tile_pool(name="small", bufs=2) as smallp,
            tc.tile_pool(name="mlp", bufs=1) as mlpp,
            tc.tile_pool(name="acc", bufs=1, space="PSUM") as accp,
            tc.tile_pool(name="pch", bufs=2, space="PSUM") as pchp,
            tc.tile_pool(name="pmlp", bufs=2, space="PSUM") as pmlpp,
            tc.tile_pool(name="dram", bufs=2, space="DRAM") as dramp,
        ):
            # ---- constants + weights to SBUF
            iota128_sb = constp.tile([128, 128], BF16, tag="iota128")
            nc.sync.dma_start(iota128_sb[:], iota128[:])
            iota80_sb = constp.tile([128, HI], BF16, tag="iota80")
            nc.sync.dma_start(iota80_sb[:], iota80[:])
            ident_sb = constp.tile([128, 128], BF16, tag="ident")
            nc.sync.dma_start(ident_sb[:], ident_bf[:])

            gcnw_sb = constp.tile([GCN_D, GCN_D], F32, tag="gcnw")
            nc.sync.dma_start(gcnw_sb[:], gcn_w[:])
            gcnb_sb = constp.tile([GCN_D, 1], F32, tag="gcnb")
            nc.sync.dma_start(gcnb_sb[:], gcn_b[:])

            w1_sb = {}
            w2_sb = {}
            w3_sb = {}
            b1_sb = {}
            b2_sb = {}
            b3_sb = {}
            for nm, (W1, W2, W3, B1, B2, B3, od) in {
                "a": (aw1, aw2, aw3, ab1, ab2, ab3, OUT_DIM),
                "c": (cw1, cw2, cw3, cb1, cb2, cb3, 1),
            }.items():
                w1_sb[nm] = constp.tile([128, H1], F32, name=f"w1_sb_{nm}", tag=f"{nm}w1")
                nc.sync.dma_start(w1_sb[nm][:], W1[:])
                w2_sb[nm] = constp.tile([128, 4 * H2], F32, name=f"w2_sb_{nm}", tag=f"{nm}w2")
                nc.sync.dma_start(w2_sb[nm][:].rearrange("p (a m) -> p a m", a=4), W2[:].rearrange("a p m -> p a m"))
                w3_sb[nm] = constp.tile([128, 8 * od], F32, name=f"w3_sb_{nm}", tag=f"{nm}w3")
                nc.sync.dma_start(w3_sb[nm][:].rearrange("p (a m) -> p a m", a=8), W3[:].rearrange("a p m -> p a m"))
                b1_sb[nm] = constp.tile([128, 4], F32, name=f"b1_sb_{nm}", tag=f"{nm}b1")
                nc.sync.dma_start(b1_sb[nm][:], B1[:])
                b2_sb[nm] = constp.tile([128, 8], F32, name=f"b2_sb_{nm}", tag=f"{nm}b2")
                nc.sync.dma_start(b2_sb[nm][:], B2[:])
                b3_sb[nm] = constp.tile([od, 1], F32, name=f"b3_sb_{nm}", tag=f"{nm}b3")
                nc.sync.dma_start(b3_sb[nm][:], B3[:])

            hT_sb = mlpp.tile([128, GPC], F32, tag="hT")
            nc.sync.dma_start(hT_sb[GCN_D:128, :], xT[:])

            if debug:
                dbg_sb = mlpp.tile([128, 256], F32, tag="dbg")

            def onehot(out_t, bn, vals2d, iota_sb, width, eng=None):
                """out_t[:, :bn*width] = onehot(vals2d[:, :bn]) against iota row."""
                vals_ap = _ap3(vals2d, [[1, bn], [0, width]])
                iota_ap = _ap3(iota_sb[:, 0:width], [[0, bn], [1, width]])
                out_ap = _ap3(
                    out_t[:, 0 : bn * width], [[width, bn], [1, width]]
                )
                (eng or nc.vector).tensor_tensor(
                    out=out_ap, in0=vals_ap, in1=iota_ap, op=OP.is_equal
                )

            # ================= per-graph pipeline =================
            for g in range(GPC):
                # ---- load edges, split hi/lo, cast to bf16
                dst_i = edgep.tile([128, EPP], I32, tag="dst_i")
                nc.sync.dma_start(
                    dst_i[:], edges[g, 1].rearrange("(p c) -> p c", p=128)
                )
                src_i = edgep.tile([128, EPP], I32, tag="src_i")
                nc.sync.dma_start(
                    src_i[:], edges[g, 0].rearrange("(p c) -> p c", p=128)
                )

                tmp_i = edgep.tile([128, EPP], I32, tag="tmp_i")
                dhi = edgep.tile([128, EPP], BF16, tag="dhi")
                dlo = edgep.tile([128, EPP], BF16, tag="dlo")
                shi = edgep.tile([128, EPP], BF16, tag="shi")
                slo = edgep.tile([128, EPP], BF16, tag="slo")
                for (idx_t, hi_t, lo_t) in ((dst_i, dhi, dlo), (src_i, shi, slo)):
                    nc.vector.tensor_scalar(
                        out=tmp_i[:], in0=idx_t[:], scalar1=7, scalar2=None,
                        op0=OP.arith_shift_right,
                    )
                    nc.vector.tensor_copy(out=hi_t[:], in_=tmp_i[:])
                    nc.vector.tensor_scalar(
                        out=tmp_i[:], in0=idx_t[:], scalar1=127, scalar2=None,
                        op0=OP.bitwise_and,
                    )
                    nc.vector.tensor_copy(out=lo_t[:], in_=tmp_i[:])

                # ---- load this graph's node features (overlaps edge phase)
                gx_sb = gxp.tile([128, HI * GCN_D], F32, tag="gx")
                nc.sync.dma_start(gx_sb[:].rearrange("p (c d) -> p c d", c=HI), gx[g].rearrange("c p d -> p c d"))

                # ---- phase B: degree histogram  deg2dT[lo, hi]
                bt_dram = dramp.tile([128, chunks * 128], BF16, tag="btd", name="bt_dram")
                ad_dram = dramp.tile([128, chunks * HI], BF16, tag="add", name="ad_dram")
                degT_ps = accp.tile([128, HI], F32, tag="degT", space="PSUM")
                k = 0
                while k < chunks:
                    bn = min(BN, chunks - k)
                    Bd = onehp.tile([128, BN * 128], BF16, tag="Bd")
                    Ad = onehp.tile([128, BN * HI], BF16, tag="Ad")
                    onehot(Bd, bn, dlo[:, k : k + bn], iota128_sb, 128)
                    onehot(Ad, bn, dhi[:, k : k + bn], iota80_sb, HI)
                    for j in range(bn):
                        nc.tensor.matmul(
                            out=degT_ps[:],
                            lhsT=Bd[:, j * 128 : (j + 1) * 128],
                            rhs=Ad[:, j * HI : (j + 1) * HI],
                            start=(k + j == 0),
                            stop=(k + j == chunks - 1),
                            skip_group_check=True,
                        )
                    # transpose Bd now and spill BT + Ad to DRAM scratch so
                    # phase D streams them back instead of rebuilding (DVE is
                    # the bottleneck; DMA engines are idle)
                    for j0 in range(0, bn, 4):
                        b4 = min(4, bn - j0)
                        BT_ps = pchp.tile([128, 4 * 128], BF16, tag="BT", space="PSUM")
                        BT_sb = onehp.tile([128, 4 * 128], BF16, tag="BTsb")
                        for q in range(b4):
                            j = j0 + q
                            nc.tensor.transpose(
                                out=BT_ps[:, q * 128 : (q + 1) * 128],
                                in_=Bd[:, j * 128 : (j + 1) * 128],
                                identity=ident_sb[:],
                            )
                        nc.scalar.activation(
                            out=BT_sb[:, 0 : b4 * 128], in_=BT_ps[:, 0 : b4 * 128],
                            func=AF.Identity,
                        )
                        nc.sync.dma_start(
                            bt_dram[:, (k + j0) * 128 : (k + j0 + b4) * 128],
                            BT_sb[:, 0 : b4 * 128],
                        )
                    nc.sync.dma_start(
                        ad_dram[:, k * HI : (k + bn) * HI], Ad[:, 0 : bn * HI]
                    )
                    k += bn

                # ---- phase C: dis = 1/sqrt(deg+1); keep f32 + bf16 copies
                deg1_sb = smallp.tile([128, HI], F32, tag="deg1")
                nc.scalar.activation(
                    out=deg1_sb[:], in_=degT_ps[:], func=AF.Identity, bias=1.0
                )
                rec_sb = smallp.tile([128, HI], F32, tag="rec")
                nc.vector.reciprocal(out=rec_sb[:], in_=deg1_sb[:])
                disT_sb = smallp.tile([128, HI], F32, tag="disT")
                nc.scalar.activation(out=disT_sb[:], in_=rec_sb[:], func=AF.Sqrt)
                disT_bf = smallp.tile([128, HI], BF16, tag="disTbf")
                nc.vector.tensor_copy(out=disT_bf[:], in_=disT_sb[:])

                # ---- phase D: dis_dst gather + t scatter  t2dT[lo, hi]
                tT_ps = accp.tile([128, HI], F32, tag="tT", space="PSUM")
                dd_f = edgep.tile([128, EPP], F32, tag="dd_f")
                k = 0
                while k < chunks:
                    bn = min(BN, chunks - k)
                    Bs = onehp.tile([128, BN * 128], BF16, tag="Bs")
                    WA = onehp.tile([128, BN * HI], BF16, tag="WA")
                    scr = onehp.tile([128, BN * HI], BF16, tag="scr")
                    G_bt = onehp.tile([128, BN * HI], BF16, tag="Gbt")
                    onehot(Bs, bn, slo[:, k : k + bn], iota128_sb, 128)
                    # stream BT back from scratch, run G matmuls 4 chunks at a time
                    for j0 in range(0, bn, 4):
                        b4 = min(4, bn - j0)
                        G_ps = pchp.tile([128, 4 * HI], F32, tag="G", space="PSUM")
                        BT_sb = onehp.tile([128, 4 * 128], BF16, tag="BTsb")
                        nc.sync.dma_start(
                            BT_sb[:, 0 : b4 * 128],
                            bt_dram[:, (k + j0) * 128 : (k + j0 + b4) * 128],
                        )
                        for q in range(b4):
                            nc.tensor.matmul(
                                out=G_ps[:, q * HI : (q + 1) * HI],
                                lhsT=BT_sb[:, q * 128 : (q + 1) * 128],
                                rhs=disT_bf[:],
                                skip_group_check=True,
                            )
                        nc.scalar.activation(
                            out=G_bt[:, j0 * HI : (j0 + b4) * HI],
                            in_=G_ps[:, 0 : b4 * HI],
                            func=AF.Identity,
                        )
                    Ad = onehp.tile([128, BN * HI], BF16, tag="Ad")
                    As = onehp.tile([128, BN * HI], BF16, tag="As")
                    nc.sync.dma_start(
                        Ad[:, 0 : bn * HI], ad_dram[:, k * HI : (k + bn) * HI]
                    )
                    onehot(As, bn, shi[:, k : k + bn], iota80_sb, HI)
                    nc.vector.tensor_tensor(
                        out=scr[:, 0 : bn * HI], in0=Ad[:, 0 : bn * HI],
                        in1=G_bt[:, 0 : bn * HI], op=OP.mult,
                    )
                    nc.vector.tensor_reduce(
                        out=dd_f[:, k : k + bn],
                        in_=_ap3(scr[:, 0 : bn * HI], [[HI, bn], [1, HI]]),
                        axis=mybir.AxisListType.X,
                        op=OP.add,
                    )
                    nc.vector.tensor_tensor(
                        out=_ap3(WA[:, 0 : bn * HI], [[HI, bn], [1, HI]]),
                        in0=_ap3(As[:, 0 : bn * HI], [[HI, bn], [1, HI]]),
                        in1=_ap3(dd_f[:, k : k + bn], [[1, bn], [0, HI]]),
                        op=OP.mult,
                    )
                    for j in range(bn):
                        nc.tensor.matmul(
                            out=tT_ps[:],
                            lhsT=Bs[:, j * 128 : (j + 1) * 128],
                            rhs=WA[:, j * HI : (j + 1) * HI],
                            start=(k + j == 0),
                            stop=(k + j == chunks - 1),
                            skip_group_check=True,
                        )
                    k += bn

                # ---- phase E: u, weighted node-sum v, g
                uT_sb = smallp.tile([128, HI], F32, tag="uT")
                nc.vector.tensor_tensor(
                    out=uT_sb[:], in0=tT_ps[:], in1=disT_sb[:], op=OP.add
                )
                nc.vector.tensor_tensor(
                    out=uT_sb[:], in0=uT_sb[:], in1=disT_sb[:], op=OP.mult
                )
                nc.vector.tensor_scalar(
                    out=uT_sb[:], in0=uT_sb[:], scalar1=1.0 / N, scalar2=None,
                    op0=OP.mult,
                )

                vT_ps = pmlpp.tile([GCN_D, 1], F32, tag="mp", space="PSUM")
                for h in range(HI):
                    nc.tensor.matmul(
                        out=vT_ps[:],
                        lhsT=gx_sb[:, h * GCN_D : (h + 1) * GCN_D],
                        rhs=uT_sb[:, h : h + 1],
                        start=(h == 0),
                        stop=(h == HI - 1),
                        skip_group_check=True,
                    )
                vT_sb = smallp.tile([GCN_D, 1], F32, tag="vT_sb")
                nc.scalar.activation(out=vT_sb[:], in_=vT_ps[:], func=AF.Identity)
                gT_ps = pmlpp.tile([GCN_D, 1], F32, tag="mp", space="PSUM")
                nc.tensor.matmul(
                    out=gT_ps[:], lhsT=gcnw_sb[:], rhs=vT_sb[:],
                    skip_group_check=True,
                )
                nc.scalar.activation(
                    out=hT_sb[0:GCN_D, g : g + 1], in_=gT_ps[:],
                    func=AF.Identity, bias=gcnb_sb[:, 0:1],
                )

                if debug:
                    if g == 0:
                        nc.scalar.activation(
                            out=dbg_sb[:, 0:HI], in_=deg1_sb[:], func=AF.Identity
                        )
                        nc.scalar.activation(
                            out=dbg_sb[:, 80:160], in_=uT_sb[:], func=AF.Identity
                        )
                        nc.scalar.activation(
                            out=dbg_sb[:, 160:240], in_=disT_sb[:], func=AF.Identity
                        )
                        nc.scalar.activation(
                            out=dbg_sb[:, 240:241], in_=dd_f[:, 0:1], func=AF.Identity
                        )

            # ================= MLP heads (both graphs batched) =================
            def lrelu(out_sb, in_ps, bias_ap):
                nc.scalar.activation(
                    out=out_sb, in_=in_ps, func=AF.Identity, bias=bias_ap
                )
                tmp = mlpp.tile([128, GPC], F32, tag="lrtmp")
                nc.vector.tensor_scalar(
                    out=tmp[: out_sb.shape[0], :], in0=out_sb, scalar1=0.01,
                    scalar2=None, op0=OP.mult,
                )
                nc.vector.tensor_tensor(
                    out=out_sb, in0=out_sb, in1=tmp[: out_sb.shape[0], :], op=OP.max
                )

            outT_parts = {}
            for nm, od in (("a", OUT_DIM), ("c", 1)):
                a1_ps = pmlpp.tile([128, 4 * GPC], F32, tag="mp", space="PSUM")
                for m in range(4):
                    nc.tensor.matmul(
                        out=a1_ps[:, m * GPC : (m + 1) * GPC],
                        lhsT=w1_sb[nm][:, m * 128 : (m + 1) * 128],
                        rhs=hT_sb[:],
                        skip_group_check=True,
                    )
                a1_sb = mlpp.tile([128, 4 * GPC], F32, tag=f"{nm}1sb")
                for m in range(4):
                    lrelu(
                        a1_sb[:, m * GPC : (m + 1) * GPC],
                        a1_ps[:, m * GPC : (m + 1) * GPC],
                        b1_sb[nm][:, m : m + 1],
                    )
                a2_ps = pmlpp.tile([128, 8 * GPC], F32, tag="mp", space="PSUM")
                for m in range(8):
                    for kk in range(4):
                        nc.tensor.matmul(
                            out=a2_ps[:, m * GPC : (m + 1) * GPC],
                            lhsT=w2_sb[nm][:, kk * H2 + m * 128 : kk * H2 + (m + 1) * 128],
                            rhs=a1_sb[:, kk * GPC : (kk + 1) * GPC],
                            start=(kk == 0),
                            stop=(kk == 3),
                            skip_group_check=True,
                        )
                a2_sb = mlpp.tile([128, 8 * GPC], F32, tag=f"{nm}2sb")
                for m in range(8):
                    lrelu(
                        a2_sb[:, m * GPC : (m + 1) * GPC],
                        a2_ps[:, m * GPC : (m + 1) * GPC],
                        b2_sb[nm][:, m : m + 1],
                    )
                a3_ps = pmlpp.tile([od, GPC], F32, tag="mp", space="PSUM")
                for kk in range(8):
                    nc.tensor.matmul(
                        out=a3_ps[:],
                        lhsT=w3_sb[nm][:, kk * od : (kk + 1) * od],
                        rhs=a2_sb[:, kk * GPC : (kk + 1) * GPC],
                        start=(kk == 0),
                        stop=(kk == 7),
                        skip_group_check=True,
                    )
                a3_sb = mlpp.tile([od, GPC], F32, tag=f"{nm}3sb")
                nc.scalar.activation(
                    out=a3_sb[:], in_=a3_ps[:], func=AF.Identity,
                    bias=b3_sb[nm][:, 0:1],
                )
                outT_parts[nm] = a3_sb

            nc.sync.dma_start(out[0:OUT_DIM, :], outT_parts["a"][:])
            nc.sync.dma_start(out[OUT_DIM : OUT_DIM + 1, :], outT_parts["c"][:])
            if debug:
                nc.sync.dma_start(dbg[:], dbg_sb[:])

    split_waits(nc)
    return nc


_NC_CACHE = {}


def _get_nc(chunks=CHUNKS, debug=DEBUG):
    key = (chunks, debug)
    if key not in _NC_CACHE:
        _NC_CACHE[key] = build_nc(chunks, debug)
    return _NC_CACHE[key]


def make_in_maps(x, gcn_x, gcn_edge_index, gcn_w, gcn_b,
                 aw1, ab1, aw2, ab2, aw3, ab3,
                 cw1, cb1, cw2, cb2, cw3, cb3):
    f32 = np.float32
    bf = ml_dtypes.bfloat16
    gxp = np.zeros((B, NPAD, GCN_D), f32)
    gxp[:, :N, :] = np.asarray(gcn_x, f32)
    gxp = gxp.reshape(B, HI, 128, GCN_D)

    shared = {
        "gcn_w": np.ascontiguousarray(np.asarray(gcn_w, f32)),
        "gcn_b": np.asarray(gcn_b, f32).reshape(GCN_D, 1),
        "aw1": np.ascontiguousarray(np.asarray(aw1, f32)),
        "aw2": np.ascontiguousarray(np.asarray(aw2, f32).reshape(4, 128, H2)),
        "aw3": np.ascontiguousarray(np.asarray(aw3, f32).reshape(8, 128, OUT_DIM)),
        "cw1": np.ascontiguousarray(np.asarray(cw1, f32)),
        "cw2": np.ascontiguousarray(np.asarray(cw2, f32).reshape(4, 128, H2)),
        "cw3": np.ascontiguousarray(np.asarray(cw3, f32).reshape(8, 128, 1)),
        "ab1": np.ascontiguousarray(np.asarray(ab1, f32).reshape(4, 128).T),
        "ab2": np.ascontiguousarray(np.asarray(ab2, f32).reshape(8, 128).T),
        "ab3": np.asarray(ab3, f32).reshape(OUT_DIM, 1),
        "cb1": np.ascontiguousarray(np.asarray(cb1, f32).reshape(4, 128).T),
        "cb2": np.ascontiguousarray(np.asarray(cb2, f32).reshape(8, 128).T),
        "cb3": np.asarray(cb3, f32).reshape(1, 1),
        "iota128": np.ascontiguousarray(
            np.broadcast_to(np.arange(128, dtype=bf), (128, 128))
        ),
        "iota80": np.ascontiguousarray(
            np.broadcast_to(np.arange(HI, dtype=bf), (128, HI))
        ),
        "ident_bf": np.eye(128, dtype=bf),
    }
    in_maps = []
    for c in range(N_CORES):
        sl = slice(c * GPC, (c + 1) * GPC)
        m = dict(shared)
        m["edges"] = np.ascontiguousarray(np.asarray(gcn_edge_index[sl], np.int32))
        m["gx"] = np.ascontiguousarray(gxp[sl])
        m["xT"] = np.ascontiguousarray(np.asarray(x[sl], f32).T)
        in_maps.append(m)
    return in_maps


def run(inputs, trace=False, chunks=CHUNKS, debug=DEBUG):
    from concourse.bass_utils import run_bass_kernel_spmd

    nc = _get_nc(chunks, debug)
    in_maps = make_in_maps(**inputs)
    res = run_bass_kernel_spmd(
        nc, in_maps, core_ids=list(range(N_CORES)), trace=trace
    )
    a = np.zeros((B, OUT_DIM), np.float32)
    cc = np.zeros((B, 1), np.float32)
    for i in range(N_CORES):
        o = res.results[i]["out"]  # [17, GPC]
        a[i * GPC : (i + 1) * GPC] = o[:OUT_DIM].T
        cc[i * GPC : (i + 1) * GPC] = o[OUT_DIM:].T
    return (a, cc), res


def kernel(**inputs):
    (a, cc), _ = run(inputs, trace=False)
    return (a, cc)


# revision 24
# speedup vs baseline: 1.4267x; 1.2855x over previous
"""Trainium2 Bass kernel for nn_A2CNetwork (GCN mean-pool + 2 MLP heads).

Self-contained: hardcodes shapes, shards batch over 8 NeuronCores
(2 graphs per core), builds one SPMD Bass/Tile graph, runs via
run_bass_kernel_spmd, reassembles full outputs.

Math (per graph), collapsing GCNConv + mean-pool algebraically:
  deg[i]  = 1 + #{e : dst_e == i}                (self-loops included)
  dis     = 1/sqrt(deg)
  t[j]    = sum_{e: src_e == j} dis[dst_e]
  u[j]    = dis[j] * (t[j] + dis[j]) / N
  v       = sum_j u[j] * gcn_x[j, :]             (weighted node sum)
  g       = v @ gcn_w + gcn_b
  h       = concat([g, x]);  two 3-layer MLP heads -> (a, c)

On device the histogram / gather / scatter run as one-hot matmuls over a
(hi=idx>>7, lo=idx&127) 80x128 decomposition: per 128-edge chunk, bf16
one-hot tiles are built with is_equal against iota tables, contracted on
the TensorEngine, and dis[dst_e] is extracted with a PE transpose + matmul
against dis2d^T, a masked multiply, and a batched reduce.
"""

import os
import sys

sys.path.insert(0, "/opt/trn_rl_repo")
os.environ.setdefault("MYCRO_LOCAL_CACHE", "1")

import numpy as np
import ml_dtypes

import concourse.bass as bass
import concourse.tile as tile
from concourse import mybir
from concourse.vector_clock import ScopedClock

# ----------------------------------------------------------------------------
# problem dims (hardcoded per spec)
B, N, E = 16, 10000, 160000
GCN_D, IN_DIM, OUT_DIM = 64, 128, 16
RAW_D = IN_DIM - GCN_D  # 64
H1, H2 = 512, 1024
N_CORES = 8
GPC = B // N_CORES  # graphs per core = 2

HI, LO = 80, 128  # node = hi*128 + lo;  hi in [0,79), lo in [0,128)
NPAD = HI * LO  # 10240
CHUNKS = E // 128  # 1250 edge chunks of 128 (edge k*? = strided layout)
EPP = E // 128  # elements per partition for edge arrays = 1250
BN = 16  # chunks per batched one-hot builder op

F32 = mybir.dt.float32
BF16 = mybir.dt.bfloat16
I32 = mybir.dt.int32
AF = mybir.ActivationFunctionType
OP = mybir.AluOpType

DEBUG = False




class SplitDrainTileContext(tile.TileContext):
    """Walrus in this image rejects >1 sync-wait on one Drain; split them."""

    MAX_WAITS = 1

    def _drain_and_barrier(self, tick_clock, wait_clock):
        import bass_rust

        drain_inst = self.nc.sync.drain()
        wait_clock.add_sem_waits(
            drain_inst.ins, ScopedClock({None: tick_clock.global_clock})
        )
        si = drain_inst.ins.sync_info
        mw = self.MAX_WAITS
        if si is not None and si.on_wait is not None and len(si.on_wait) > mw:
            waits = list(si.on_wait)
            si.on_wait = waits[:mw]
            rest = waits[mw:]
            while rest:
                d2 = self.nc.sync.drain()
                if d2.ins.sync_info is None:
                    d2.ins.sync_info = bass_rust.SyncInfo(
                        on_wait=rest[:mw], on_update=[]
                    )
                else:
                    d2.ins.sync_info.on_wait = rest[:mw]
                rest = rest[mw:]
        self.nc.all_engine_barrier()
        assert self.sems is not None
        popped = self.nc._tile_sem_poison_stack.pop()
        assert popped is self._sem_poison
        self.nc.clear_and_free_semaphores(list(self.sems.allocated().values()))
        self.nc.all_engine_barrier()


def _ap3(ap, steps):
    """Manual 3D AP on top of a 2D slice: steps = [[s1,c1],[s2,c2]] free dims."""
    return bass.AP(ap.tensor, ap.offset, [list(ap.ap[0])] + [list(s) for s in steps])


def split_waits(nc, max_waits=1):
    """This walrus build encodes at most one sync-wait per instruction.

    For any instruction carrying more, move the extras onto same-engine NOPs
    inserted immediately before it (same sequencer => still ordered).
    """
    import bass_rust

    for bb in nc.main_func.blocks:
        il = bb.instructions
        extra_total = 0
        newl = []
        for ins in il:
            si = ins.sync_info
            if si is not None and si.on_wait is not None and len(si.on_wait) > max_waits:
                waits = list(si.on_wait)
                si.on_wait = waits[:max_waits]
                rest = waits[max_waits:]
                eng = nc.engines[ins.engine]
                while rest:
                    nop = eng.nop()
                    # relocate: eng.nop() appended to the current bb tail
                    src_list = nc.cur_bb.bb.instructions
                    assert src_list[-1] is nop.ins
                    src_list.pop()
                    nop.ins.sync_info = bass_rust.SyncInfo(
                        on_wait=rest[:max_waits], on_update=[]
                    )
                    newl.append(nop.ins)
                    rest = rest[max_waits:]
                    extra_total += 1
            newl.append(ins)
        il[:] = newl


def build_nc(chunks=CHUNKS, debug=DEBUG):
    nc = bass.Bass()

    # ---- dram parameters (per-core shards / replicated weights / constants)
    edges = nc.declare_dram_parameter("edges", [GPC, 2, E], I32, isOutput=False)
    gx = nc.declare_dram_parameter("gx", [GPC, HI, 128, GCN_D], F32, isOutput=False)
    xT = nc.declare_dram_parameter("xT", [RAW_D, GPC], F32, isOutput=False)
    gcn_w = nc.declare_dram_parameter("gcn_w", [GCN_D, GCN_D], F32, isOutput=False)
    gcn_b = nc.declare_dram_parameter("gcn_b", [GCN_D, 1], F32, isOutput=False)
    aw1 = nc.declare_dram_parameter("aw1", [IN_DIM, H1], F32, isOutput=False)
    aw2 = nc.declare_dram_parameter("aw2", [4, 128, H2], F32, isOutput=False)
    aw3 = nc.declare_dram_parameter("aw3", [8, 128, OUT_DIM], F32, isOutput=False)
    cw1 = nc.declare_dram_parameter("cw1", [IN_DIM, H1], F32, isOutput=False)
    cw2 = nc.declare_dram_parameter("cw2", [4, 128, H2], F32, isOutput=False)
    cw3 = nc.declare_dram_parameter("cw3", [8, 128, 1], F32, isOutput=False)
    ab1 = nc.declare_dram_parameter("ab1", [128, 4], F32, isOutput=False)
    ab2 = nc.declare_dram_parameter("ab2", [128, 8], F32, isOutput=False)
    ab3 = nc.declare_dram_parameter("ab3", [OUT_DIM, 1], F32, isOutput=False)
    cb1 = nc.declare_dram_parameter("cb1", [128, 4], F32, isOutput=False)
    cb2 = nc.declare_dram_parameter("cb2", [128, 8], F32, isOutput=False)
    cb3 = nc.declare_dram_parameter("cb3", [1, 1], F32, isOutput=False)
    iota128 = nc.declare_dram_parameter("iota128", [128, 128], BF16, isOutput=False)
    iota80 = nc.declare_dram_parameter("iota80", [128, HI], BF16, isOutput=False)
    ident_bf = nc.declare_dram_parameter("ident_bf", [128, 128], BF16, isOutput=False)
    out = nc.declare_dram_parameter("out", [OUT_DIM + 1, GPC], F32, isOutput=True)
    if debug:
        dbg = nc.declare_dram_parameter("dbg", [128, 256], F32, isOutput=True)

    with SplitDrainTileContext(nc) as tc:
        with (
            tc.tile_pool(name="const", bufs=1) as constp,
            tc.tile_pool(name="edge", bufs=2) as edgep,
            tc.tile_pool(name="oneh", bufs=2) as onehp,
            tc.tile_pool(name="gxp", bufs=1) as gxp,
            tc.tile_pool(name="small", bufs=2) as smallp,
            tc.tile_pool(name="mlp", bufs=1) as mlpp,
            tc.tile_pool(name="acc", bufs=1, space="PSUM") as accp,
            tc.tile_pool(name="pch", bufs=2, space="PSUM") as pchp,
            tc.tile_pool(name="pmlp", bufs=2, space="PSUM") as pmlpp,
            tc.tile_pool(name="dram", bufs=2, space="DRAM") as dramp,
        ):
            # ---- constants + weights to SBUF
            iota128_sb = constp.tile([128, 128], BF16, tag="iota128")
            nc.sync.dma_start(iota128_sb[:], iota128[:])
            iota80_sb = constp.tile([128, HI], BF16, tag="iota80")
            nc.sync.dma_start(iota80_sb[:], iota80[:])
            ident_sb = constp.tile([128, 128], BF16, tag="ident")
            nc.sync.dma_start(ident_sb[:], ident_bf[:])

            gcnw_sb = constp.tile([GCN_D, GCN_D], F32, tag="gcnw")
            nc.sync.dma_start(gcnw_sb[:], gcn_w[:])
            gcnb_sb = constp.tile([GCN_D, 1], F32, tag="gcnb")
            nc.sync.dma_start(gcnb_sb[:], gcn_b[:])

            w1_sb = {}
            w2_sb = {}
            w3_sb = {}
            b1_sb = {}
            b2_sb = {}
            b3_sb = {}
            for nm, (W1, W2, W3, B1, B2, B3, od) in {
                "a": (aw1, aw2, aw3, ab1, ab2, ab3, OUT_DIM),
                "c": (cw1, cw2, cw3, cb1, cb2, cb3, 1),
            }.items():
                w1_sb[nm] = constp.tile([128, H1], F32, name=f"w1_sb_{nm}", tag=f"{nm}w1")
                nc.sync.dma_start(w1_sb[nm][:], W1[:])
                w2_sb[nm] = constp.tile([128, 4 * H2], F32, name=f"w2_sb_{nm}", tag=f"{nm}w2")
                nc.sync.dma_start(w2_sb[nm][:].rearrange("p (a m) -> p a m", a=4), W2[:].rearrange("a p m -> p a m"))
                w3_sb[nm] = constp.tile([128, 8 * od], F32, name=f"w3_sb_{nm}", tag=f"{nm}w3")
                nc.sync.dma_start(w3_sb[nm][:].rearrange("p (a m) -> p a m", a=8), W3[:].rearrange("a p m -> p a m"))
                b1_sb[nm] = constp.tile([128, 4], F32, name=f"b1_sb_{nm}", tag=f"{nm}b1")
                nc.sync.dma_start(b1_sb[nm][:], B1[:])
                b2_sb[nm] = constp.tile([128, 8], F32, name=f"b2_sb_{nm}", tag=f"{nm}b2")
                nc.sync.dma_start(b2_sb[nm][:], B2[:])
                b3_sb[nm] = constp.tile([od, 1], F32, name=f"b3_sb_{nm}", tag=f"{nm}b3")
                nc.sync.dma_start(b3_sb[nm][:], B3[:])

            hT_sb = mlpp.tile([128, GPC], F32, tag="hT")
            nc.sync.dma_start(hT_sb[GCN_D:128, :], xT[:])

            if debug:
                dbg_sb = mlpp.tile([128, 256], F32, tag="dbg")

            def onehot(out_t, bn, vals2d, iota_sb, width, eng=None):
                """out_t[:, :bn*width] = onehot(vals2d[:, :bn]) against iota row."""
                vals_ap = _ap3(vals2d, [[1, bn], [0, width]])
                iota_ap = _ap3(iota_sb[:, 0:width], [[0, bn], [1, width]])
                out_ap = _ap3(
                    out_t[:, 0 : bn * width], [[width, bn], [1, width]]
                )
                (eng or nc.vector).tensor_tensor(
                    out=out_ap, in0=vals_ap, in1=iota_ap, op=OP.is_equal
                )

            # ================= per-graph pipeline =================
            for g in range(GPC):
                # ---- load edges, split hi/lo, cast to bf16
                dst_i = edgep.tile([128, EPP], I32, tag="dst_i")
                nc.sync.dma_start(
                    dst_i[:], edges[g, 1].rearrange("(p c) -> p c", p=128)
                )
                src_i = edgep.tile([128, EPP], I32, tag="src_i")
                nc.sync.dma_start(
                    src_i[:], edges[g, 0].rearrange("(p c) -> p c", p=128)
                )

                tmp_i = edgep.tile([128, EPP], I32, tag="tmp_i")
                dhi = edgep.tile([128, EPP], BF16, tag="dhi")
                dlo = edgep.tile([128, EPP], BF16, tag="dlo")
                shi = edgep.tile([128, EPP], BF16, tag="shi")
                slo = edgep.tile([128, EPP], BF16, tag="slo")
                for (idx_t, hi_t, lo_t) in ((dst_i, dhi, dlo), (src_i, shi, slo)):
                    nc.vector.tensor_scalar(
                        out=tmp_i[:], in0=idx_t[:], scalar1=7, scalar2=None,
                        op0=OP.arith_shift_right,
                    )
                    nc.vector.tensor_copy(out=hi_t[:], in_=tmp_i[:])
                    nc.vector.tensor_scalar(
                        out=tmp_i[:], in0=idx_t[:], scalar1=127, scalar2=None,
                        op0=OP.bitwise_and,
                    )
                    nc.vector.tensor_copy(out=lo_t[:], in_=tmp_i[:])

                # ---- load this graph's node features (overlaps edge phase)
                gx_sb = gxp.tile([128, HI * GCN_D], F32, tag="gx")
                nc.sync.dma_start(gx_sb[:].rearrange("p (c d) -> p c d", c=HI), gx[g].rearrange("c p d -> p c d"))

                # ---- phase B: degree histogram  deg2dT[lo, hi]
                bt_dram = dramp.tile([128, chunks * 128], BF16, tag="btd", name="bt_dram")
                ad_dram = dramp.tile([128, chunks * HI], BF16, tag="add", name="ad_dram")
                degT_ps = accp.tile([128, HI], F32, tag="degT", space="PSUM")
                k = 0
                while k < chunks:
                    bn = min(BN, chunks - k)
                    Bd = onehp.tile([128, BN * 128], BF16, tag="Bd")
                    Ad = onehp.tile([128, BN * HI], BF16, tag="Ad")
                    onehot(Bd, bn, dlo[:, k : k + bn], iota128_sb, 128)
                    onehot(Ad, bn, dhi[:, k : k + bn], iota80_sb, HI)
                    for j in range(bn):
                        nc.tensor.matmul(
                            out=degT_ps[:],
                            lhsT=Bd[:, j * 128 : (j + 1) * 128],
                            rhs=Ad[:, j * HI : (j + 1) * HI],
                            start=(k + j == 0),
                            stop=(k + j == chunks - 1),
                            skip_group_check=True,
                        )
                    # transpose Bd now and spill BT + Ad to DRAM scratch for
                    # EVEN batches only: phase D streams those back while its
                    # DVE rebuilds the odd ones (build rate ~= DMA move rate,
                    # so split the work across both engines)
                    if (k // BN) % 2 == 1:
                        k += bn
                        continue
                    for j0 in range(0, bn, 4):
                        b4 = min(4, bn - j0)
                        BT_ps = pchp.tile([128, 4 * 128], BF16, tag="BT", space="PSUM")
                        BT_sb = onehp.tile([128, 4 * 128], BF16, tag="BTsb")
                        for q in range(b4):
                            j = j0 + q
                            nc.tensor.transpose(
                                out=BT_ps[:, q * 128 : (q + 1) * 128],
                                in_=Bd[:, j * 128 : (j + 1) * 128],
                                identity=ident_sb[:],
                            )
                        nc.scalar.activation(
                            out=BT_sb[:, 0 : b4 * 128], in_=BT_ps[:, 0 : b4 * 128],
                            func=AF.Identity,
                        )
                        nc.sync.dma_start(
                            bt_dram[:, (k + j0) * 128 : (k + j0 + b4) * 128],
                            BT_sb[:, 0 : b4 * 128],
                        )
                    nc.sync.dma_start(
                        ad_dram[:, k * HI : (k + bn) * HI], Ad[:, 0 : bn * HI]
                    )
                    k += bn

                # ---- phase C: dis = 1/sqrt(deg+1); keep f32 + bf16 copies
                deg1_sb = smallp.tile([128, HI], F32, tag="deg1")
                nc.scalar.activation(
                    out=deg1_sb[:], in_=degT_ps[:], func=AF.Identity, bias=1.0
                )
                rec_sb = smallp.tile([128, HI], F32, tag="rec")
                nc.vector.reciprocal(out=rec_sb[:], in_=deg1_sb[:])
                disT_sb = smallp.tile([128, HI], F32, tag="disT")
                nc.scalar.activation(out=disT_sb[:], in_=rec_sb[:], func=AF.Sqrt)
                disT_bf = smallp.tile([128, HI], BF16, tag="disTbf")
                nc.vector.tensor_copy(out=disT_bf[:], in_=disT_sb[:])

                # ---- phase D: dis_dst gather + t scatter  t2dT[lo, hi]
                tT_ps = accp.tile([128, HI], F32, tag="tT", space="PSUM")
                dd_f = edgep.tile([128, EPP], F32, tag="dd_f")
                k = 0
                while k < chunks:
                    bn = min(BN, chunks - k)
                    Bs = onehp.tile([128, BN * 128], BF16, tag="Bs")
                    WA = onehp.tile([128, BN * HI], BF16, tag="WA")
                    scr = onehp.tile([128, BN * HI], BF16, tag="scr")
                    G_bt = onehp.tile([128, BN * HI], BF16, tag="Gbt")
                    onehot(Bs, bn, slo[:, k : k + bn], iota128_sb, 128)
                    spilled = (k // BN) % 2 == 0
                    Ad = onehp.tile([128, BN * HI], BF16, tag="Ad")
                    if spilled:
                        nc.sync.dma_start(
                            Ad[:, 0 : bn * HI], ad_dram[:, k * HI : (k + bn) * HI]
                        )
                    else:
                        Bd = onehp.tile([128, BN * 128], BF16, tag="Bd")
                        onehot(Bd, bn, dlo[:, k : k + bn], iota128_sb, 128)
                        onehot(Ad, bn, dhi[:, k : k + bn], iota80_sb, HI)
                    for j0 in range(0, bn, 4):
                        b4 = min(4, bn - j0)
                        G_ps = pchp.tile([128, 4 * HI], F32, tag="G", space="PSUM")
                        BT_sb = onehp.tile([128, 4 * 128], BF16, tag="BTsb")
                        if spilled:
                            nc.sync.dma_start(
                                BT_sb[:, 0 : b4 * 128],
                                bt_dram[:, (k + j0) * 128 : (k + j0 + b4) * 128],
                            )
                        else:
                            BT_ps = pchp.tile(
                                [128, 4 * 128], BF16, tag="BT", space="PSUM"
                            )
                            for q in range(b4):
                                j = j0 + q
                                nc.tensor.transpose(
                                    out=BT_ps[:, q * 128 : (q + 1) * 128],
                                    in_=Bd[:, j * 128 : (j + 1) * 128],
                                    identity=ident_sb[:],
                                )
                            nc.scalar.activation(
                                out=BT_sb[:, 0 : b4 * 128],
                                in_=BT_ps[:, 0 : b4 * 128],
                                func=AF.Identity,
                            )
                        for q in range(b4):
                            nc.tensor.matmul(
                                out=G_ps[:, q * HI : (q + 1) * HI],
                                lhsT=BT_sb[:, q * 128 : (q + 1) * 128],
                                rhs=disT_bf[:],
                                skip_group_check=True,
                            )
                        nc.scalar.activation(
                            out=G_bt[:, j0 * HI : (j0 + b4) * HI],
                            in_=G_ps[:, 0 : b4 * HI],
                            func=AF.Identity,
                        )
                    As = onehp.tile([128, BN * HI], BF16, tag="As")
                    onehot(As, bn, shi[:, k : k + bn], iota80_sb, HI)
                    nc.vector.tensor_tensor(
                        out=scr[:, 0 : bn * HI], in0=Ad[:, 0 : bn * HI],
                        in1=G_bt[:, 0 : bn * HI], op=OP.mult,
                    )
                    nc.vector.tensor_reduce(
                        out=dd_f[:, k : k + bn],
                        in_=_ap3(scr[:, 0 : bn * HI], [[HI, bn], [1, HI]]),
                        axis=mybir.AxisListType.X,
                        op=OP.add,
                    )
                    nc.vector.tensor_tensor(
                        out=_ap3(WA[:, 0 : bn * HI], [[HI, bn], [1, HI]]),
                        in0=_ap3(As[:, 0 : bn * HI], [[HI, bn], [1, HI]]),
                        in1=_ap3(dd_f[:, k : k + bn], [[1, bn], [0, HI]]),
                        op=OP.mult,
                    )
                    for j in range(bn):
                        nc.tensor.matmul(
                            out=tT_ps[:],
                            lhsT=Bs[:, j * 128 : (j + 1) * 128],
                            rhs=WA[:, j * HI : (j + 1) * HI],
                            start=(k + j == 0),
                            stop=(k + j == chunks - 1),
                            skip_group_check=True,
                        )
                    k += bn

                # ---- phase E: u, weighted node-sum v, g
                uT_sb = smallp.tile([128, HI], F32, tag="uT")
                nc.vector.tensor_tensor(
                    out=uT_sb[:], in0=tT_ps[:], in1=disT_sb[:], op=OP.add
                )
                nc.vector.tensor_tensor(
                    out=uT_sb[:], in0=uT_sb[:], in1=disT_sb[:], op=OP.mult
                )
                nc.vector.tensor_scalar(
                    out=uT_sb[:], in0=uT_sb[:], scalar1=1.0 / N, scalar2=None,
                    op0=OP.mult,
                )

                vT_ps = pmlpp.tile([GCN_D, 1], F32, tag="mp", space="PSUM")
                for h in range(HI):
                    nc.tensor.matmul(
                        out=vT_ps[:],
                        lhsT=gx_sb[:, h * GCN_D : (h + 1) * GCN_D],
                        rhs=uT_sb[:, h : h + 1],
                        start=(h == 0),
                        stop=(h == HI - 1),
                        skip_group_check=True,
                    )
                vT_sb = smallp.tile([GCN_D, 1], F32, tag="vT_sb")
                nc.scalar.activation(out=vT_sb[:], in_=vT_ps[:], func=AF.Identity)
                gT_ps = pmlpp.tile([GCN_D, 1], F32, tag="mp", space="PSUM")
                nc.tensor.matmul(
                    out=gT_ps[:], lhsT=gcnw_sb[:], rhs=vT_sb[:],
                    skip_group_check=True,
                )
                nc.scalar.activation(
                    out=hT_sb[0:GCN_D, g : g + 1], in_=gT_ps[:],
                    func=AF.Identity, bias=gcnb_sb[:, 0:1],
                )

                if debug:
                    if g == 0:
                        nc.scalar.activation(
                            out=dbg_sb[:, 0:HI], in_=deg1_sb[:], func=AF.Identity
                        )
                        nc.scalar.activation(
                            out=dbg_sb[:, 80:160], in_=uT_sb[:], func=AF.Identity
                        )
                        nc.scalar.activation(
                            out=dbg_sb[:, 160:240], in_=disT_sb[:], func=AF.Identity
                        )
                        nc.scalar.activation(
                            out=dbg_sb[:, 240:241], in_=dd_f[:, 0:1], func=AF.Identity
                        )

            # ================= MLP heads (both graphs batched) =================
            def lrelu(out_sb, in_ps, bias_ap):
                nc.scalar.activation(
                    out=out_sb, in_=in_ps, func=AF.Identity, bias=bias_ap
                )
                tmp = mlpp.tile([128, GPC], F32, tag="lrtmp")
                nc.vector.tensor_scalar(
                    out=tmp[: out_sb.shape[0], :], in0=out_sb, scalar1=0.01,
                    scalar2=None, op0=OP.mult,
                )
                nc.vector.tensor_tensor(
                    out=out_sb, in0=out_sb, in1=tmp[: out_sb.shape[0], :], op=OP.max
                )

            outT_parts = {}
            for nm, od in (("a", OUT_DIM), ("c", 1)):
                a1_ps = pmlpp.tile([128, 4 * GPC], F32, tag="mp", space="PSUM")
                for m in range(4):
                    nc.tensor.matmul(
                        out=a1_ps[:, m * GPC : (m + 1) * GPC],
                        lhsT=w1_sb[nm][:, m * 128 : (m + 1) * 128],
                        rhs=hT_sb[:],
                        skip_group_check=True,
                    )
                a1_sb = mlpp.tile([128, 4 * GPC], F32, tag=f"{nm}1sb")
                for m in range(4):
                    lrelu(
                        a1_sb[:, m * GPC : (m + 1) * GPC],
                        a1_ps[:, m * GPC : (m + 1) * GPC],
                        b1_sb[nm][:, m : m + 1],
                    )
                a2_ps = pmlpp.tile([128, 8 * GPC], F32, tag="mp", space="PSUM")
                for m in range(8):
                    for kk in range(4):
                        nc.tensor.matmul(
                            out=a2_ps[:, m * GPC : (m + 1) * GPC],
                            lhsT=w2_sb[nm][:, kk * H2 + m * 128 : kk * H2 + (m + 1) * 128],
                            rhs=a1_sb[:, kk * GPC : (kk + 1) * GPC],
                            start=(kk == 0),
                            stop=(kk == 3),
                            skip_group_check=True,
                        )
                a2_sb = mlpp.tile([128, 8 * GPC], F32, tag=f"{nm}2sb")
                for m in range(8):
                    lrelu(
                        a2_sb[:, m * GPC : (m + 1) * GPC],
                        a2_ps[:, m * GPC : (m + 1) * GPC],
                        b2_sb[nm][:, m : m + 1],
                    )
                a3_ps = pmlpp.tile([od, GPC], F32, tag="mp", space="PSUM")
                for kk in range(8):
                    nc.tensor.matmul(
                        out=a3_ps[:],
                        lhsT=w3_sb[nm][:, kk * od : (kk + 1) * od],
                        rhs=a2_sb[:, kk * GPC : (kk + 1) * GPC],
                        start=(kk == 0),
                        stop=(kk == 7),
                        skip_group_check=True,
                    )
                a3_sb = mlpp.tile([od, GPC], F32, tag=f"{nm}3sb")
                nc.scalar.activation(
                    out=a3_sb[:], in_=a3_ps[:], func=AF.Identity,
                    bias=b3_sb[nm][:, 0:1],
                )
                outT_parts[nm] = a3_sb

            nc.sync.dma_start(out[0:OUT_DIM, :], outT_parts["a"][:])
            nc.sync.dma_start(out[OUT_DIM : OUT_DIM + 1, :], outT_parts["c"][:])
            if debug:
                nc.sync.dma_start(dbg[:], dbg_sb[:])

    split_waits(nc)
    return nc


_NC_CACHE = {}


def _get_nc(chunks=CHUNKS, debug=DEBUG):
    key = (chunks, debug)
    if key not in _NC_CACHE:
        _NC_CACHE[key] = build_nc(chunks, debug)
    return _NC_CACHE[key]


def make_in_maps(x, gcn_x, gcn_edge_index, gcn_w, gcn_b,
                 aw1, ab1, aw2, ab2, aw3, ab3,
                 cw1, cb1, cw2, cb2, cw3, cb3):
    f32 = np.float32
    bf = ml_dtypes.bfloat16
    gxp = np.zeros((B, NPAD, GCN_D), f32)
    gxp[:, :N, :] = np.asarray(gcn_x, f32)
    gxp = gxp.reshape(B, HI, 128, GCN_D)

    shared = {
        "gcn_w": np.ascontiguousarray(np.asarray(gcn_w, f32)),
        "gcn_b": np.asarray(gcn_b, f32).reshape(GCN_D, 1),
        "aw1": np.ascontiguousarray(np.asarray(aw1, f32)),
        "aw2": np.ascontiguousarray(np.asarray(aw2, f32).reshape(4, 128, H2)),
        "aw3": np.ascontiguousarray(np.asarray(aw3, f32).reshape(8, 128, OUT_DIM)),
        "cw1": np.ascontiguousarray(np.asarray(cw1, f32)),
        "cw2": np.ascontiguousarray(np.asarray(cw2, f32).reshape(4, 128, H2)),
        "cw3": np.ascontiguousarray(np.asarray(cw3, f32).reshape(8, 128, 1)),
        "ab1": np.ascontiguousarray(np.asarray(ab1, f32).reshape(4, 128).T),
        "ab2": np.ascontiguousarray(np.asarray(ab2, f32).reshape(8, 128).T),
        "ab3": np.asarray(ab3, f32).reshape(OUT_DIM, 1),
        "cb1": np.ascontiguousarray(np.asarray(cb1, f32).reshape(4, 128).T),
        "cb2": np.ascontiguousarray(np.asarray(cb2, f32).reshape(8, 128).T),
        "cb3": np.asarray(cb3, f32).reshape(1, 1),
        "iota128": np.ascontiguousarray(
            np.broadcast_to(np.arange(128, dtype=bf), (128, 128))
        ),
        "iota80": np.ascontiguousarray(
            np.broadcast_to(np.arange(HI, dtype=bf), (128, HI))
        ),
        "ident_bf": np.eye(128, dtype=bf),
    }
    in_maps = []
    for c in range(N_CORES):
        sl = slice(c * GPC, (c + 1) * GPC)
        m = dict(shared)
        m["edges"] = np.ascontiguousarray(np.asarray(gcn_edge_index[sl], np.int32))
        m["gx"] = np.ascontiguousarray(gxp[sl])
        m["xT"] = np.ascontiguousarray(np.asarray(x[sl], f32).T)
        in_maps.append(m)
    return in_maps


def run(inputs, trace=False, chunks=CHUNKS, debug=DEBUG):
    from concourse.bass_utils import run_bass_kernel_spmd

    nc = _get_nc(chunks, debug)
    in_maps = make_in_maps(**inputs)
    res = run_bass_kernel_spmd(
        nc, in_maps, core_ids=list(range(N_CORES)), trace=trace
    )
    a = np.zeros((B, OUT_DIM), np.float32)
    cc = np.zeros((B, 1), np.float32)
    for i in range(N_CORES):
        o = res.results[i]["out"]  # [17, GPC]
        a[i * GPC : (i + 1) * GPC] = o[:OUT_DIM].T
        cc[i * GPC : (i + 1) * GPC] = o[OUT_DIM:].T
    return (a, cc), res


def kernel(**inputs):
    (a, cc), _ = run(inputs, trace=False)
    return (a, cc)
